# revision 32
# baseline (speedup 1.0000x reference)
"""BPNet GNN message-passing kernel for 8 Trainium2 NeuronCores.

Strategy (forced by this image: no extended-GPSIMD ucode, no indirect DMA —
both crash the device, verified experimentally; only static DMA + PE + DVE/ACT
work):
  - Node-sharded output with a data-driven node->(core, chunk) REMAP: nodes
    are redistributed so that per-(chunk q, slot i) incidence-group sizes pack
    tightly into 128-pair tiles (heavy-degree nodes concentrated in q=3, which
    gets more tiles; q=0..2 groups fit 4 tiles each). Every (edge,slot) pair
    is routed (host-side marshaling) to the owner core of its target node.
  - Host packs, per core, a position stream of pairs grouped by (q, i) with
    per-group tile counts (shared across cores). All per-pair data is baked
    into dense device tensors:
      xab  [128, L]  bf16 : one-hot-placed gathered node features + type
                            indicator for the pair's two COMPANION slots
                            (K-dim one-hot folds the per-edge weight selection
                            and bias into one fixed matmul).
      msel [128,T*64] fp8  : per-pair output-type selection mask, (d,tau)
                            column order (col 4d+tau); 0/1 exact in fp8
      oneh [128,T*128]fp8  : per-tile one-hot incidence (lane -> node row)
      cmat/bmat           : per-(slot,type)-group incidence counts + ho_bias
                            rows; bias enters via tiny matmuls.
  - Device pipeline, chunk ch = 512 pair-columns = 4 tiles:
      PE:   pa = w1a^T @ xab[ch], pb = w1b^T @ xab[ch]          (psum)
      ACT:  rb = relu(pb)                                        (sbuf bf16)
      DVE:  fact[ch] = max(pa,0) * rb    (fused scalar_tensor_tensor)
      PE:   pb2[k] = fact_t^T @ w2[:, i_t, :]  (4 tiles -> one psum block)
      ACT/DVE: pc = copy(pb2)  (Pool cannot touch PSUM; alternate engines)
      POOL: sel = pc * msel[ch]                                  (sbuf bf16)
      PE:   pn[:, q, :, :] += oneh_t^T @ sel_t   (psum wide accumulate over
            (d,tau); type-select deferred to a per-bank final tau-reduce)
      PE:   pn[:, q, :, :] += cmat_q^T @ bmat    (bias via group counts)
      DVE:  oc = reduce_tau(pn)  -> [128, 64] f32 -> DMA out (per pn bank)
    Stage-2a/2b emission lags stage-1 by 2/5 chunks so the PE stream never
    waits on the copy->mult chain (PE p-state: clock doubles only after ~3us
    of continuous busy).
  - Output per core: [128, 64] f32 = remapped nodes [row, 16q+d]; unshard =
    permutation scatter on host.
"""

import numpy as np
import ml_dtypes

N, E, ORDER, D, RANK = 4096, 16384, 3, 13, 128
NP_ = ORDER + 1  # 4 types
NCORES = 8

bf16 = ml_dtypes.bfloat16
f8 = ml_dtypes.float8_e4m3

_COMPILED = {}  # (tile_q, tile_i) -> nc


def _pieces(T):
    """Shared host/device derivation of DMA piece splits for a T-tile layout.
    Returns (xab piece sizes in chunks, msel piece sizes in tiles (4-aligned),
    oneh piece sizes in tiles)."""
    NCH = T // 4
    a = max(1, (NCH - 2) // 3)
    xp = [2, a, a, NCH - 2 - 2 * a]
    mt0 = ((T // 2) + 3) // 4 * 4
    mts = [mt0, T - mt0]
    ot = T // 4
    ots = [ot, ot, ot, T - 3 * ot]
    return xp, mts, ots


def _build_program(tile_q, tile_i):
    import concourse.bacc as bacc
    import concourse.tile as tile
    from concourse import mybir

    T = len(tile_q)
    L = 128 * T
    NCH = T // 4
    XP, MTS, OTS = _pieces(T)
    XS = np.cumsum([0] + XP).tolist()      # xab piece start chunk
    MS = np.cumsum([0] + MTS).tolist()     # msel piece start tile
    OS = np.cumsum([0] + OTS).tolist()     # oneh piece start tile
    bankB_first = min(t for t in range(T) if tile_q[t] >= 2)
    reduceA_t = max(t for t in range(T) if tile_q[t] <= 1)
    reduceB_t = T - 1

    def piece_of(t, starts):
        j = max(k for k in range(len(starts) - 1) if starts[k] <= t)
        return j, t - starts[j]

    nc = bacc.Bacc("TRN2", target_bir_lowering=False, debug=False,
                   num_devices=NCORES, enable_partition_id=False)
    BF, F32 = mybir.dt.bfloat16, mybir.dt.float32
    F8 = mybir.dt.float8e4
    Relu = mybir.ActivationFunctionType.Relu
    Copy = mybir.ActivationFunctionType.Copy
    mx, mult, add = (mybir.AluOpType.max, mybir.AluOpType.mult,
                     mybir.AluOpType.add)

    wpack = nc.dram_tensor("wpack", [128, 448], BF, kind="ExternalInput").ap()
    cpack = nc.dram_tensor("cpack", [12, 576], BF, kind="ExternalInput").ap()
    xabs = [nc.dram_tensor(f"xab{j}", [128, 512 * XP[j]], BF,
                           kind="ExternalInput").ap() for j in range(len(XP))]
    msels = [nc.dram_tensor(f"msel{j}", [128, MTS[j] * 64], F8,
                            kind="ExternalInput").ap()
             for j in range(len(MTS))]
    onehs = [nc.dram_tensor(f"oneh{j}", [128, OTS[j] * 128], F8,
                            kind="ExternalInput").ap()
             for j in range(len(OTS))]
    out = nc.dram_tensor("out", [128, 64], F32, kind="ExternalOutput").ap()

    with tile.TileContext(nc) as tc:
        with tc.tile_pool(name="inp", bufs=1) as inp, \
             tc.tile_pool(name="work", bufs=1) as work, \
             tc.tile_pool(name="rbp", bufs=3) as rbp, \
             tc.tile_pool(name="selp", bufs=10) as selp, \
             tc.tile_pool(name="ps1", bufs=4, space="PSUM") as ps1, \
             tc.tile_pool(name="ps2", bufs=3, space="PSUM") as ps2, \
             tc.tile_pool(name="psn", bufs=1, space="PSUM") as psn:
            wpack_sb = inp.tile([128, 448], BF, tag="wpack")
            cpack_sb = inp.tile([12, 576], BF, tag="cpack")
            xab_sbs = [inp.tile([128, 512 * XP[j]], BF, tag=f"xab{j}",
                                name=f"xab{j}_sb") for j in range(len(XP))]
            msel_sbs = [inp.tile([128, MTS[j], 64], F8, tag=f"msel{j}",
                                 name=f"msel{j}_sb") for j in range(len(MTS))]
            oneh_sbs = [inp.tile([128, OTS[j] * 128], F8, tag=f"oneh{j}",
                                 name=f"oneh{j}_sb") for j in range(len(OTS))]

            # DMA issue order == expected consumption order, all on sync so
            # per-queue FIFO keeps bandwidth priority aligned with need time.
            nc.sync.dma_start(wpack_sb[:], wpack[:])
            nc.sync.dma_start(xab_sbs[0][:], xabs[0][:])
            nc.sync.dma_start(xab_sbs[1][:], xabs[1][:])
            nc.sync.dma_start(oneh_sbs[0][:], onehs[0][:])
            nc.sync.dma_start(cpack_sb[:], cpack[:])
            nc.sync.dma_start(msel_sbs[0][:, :, :], msels[0][:])
            nc.sync.dma_start(oneh_sbs[1][:], onehs[1][:])
            nc.sync.dma_start(xab_sbs[2][:], xabs[2][:])
            nc.sync.dma_start(oneh_sbs[2][:], onehs[2][:])
            nc.sync.dma_start(msel_sbs[1][:, :, :], msels[1][:])
            nc.sync.dma_start(xab_sbs[3][:], xabs[3][:])
            nc.sync.dma_start(oneh_sbs[3][:], onehs[3][:])

            w1a = wpack_sb[:, 0:128]
            w1b = wpack_sb[:, 128:256]
            # w2 columns in (d, tau) order: col 4d+tau
            w2s = [wpack_sb[:, 256 + 64 * i:256 + 64 * (i + 1)]
                   for i in range(3)]
            cmats = [cpack_sb[:, 128 * q:128 * (q + 1)] for q in range(4)]
            bmat = cpack_sb[:, 512:576]

            fact = work.tile([128, L], BF, tag="fact")
            oc = work.tile([128, 4, 16], F32, tag="oc")
            # node psum split across 2 banks (q0,q1 | q2,q3) so the per-half
            # tau-reduce never blocks later segsum matmuls (no WAR on PE)
            pns = [psn.tile([128, 2, 16, 4], F32, tag="pn", name=f"pn{h}")
                   for h in range(2)]

            def stage1(ch):
                pa = ps1.tile([128, 512], F32, tag="p1")
                pb = ps1.tile([128, 512], F32, tag="p1")
                xp, xo = piece_of(ch, XS)
                lsl = slice(512 * xo, 512 * (xo + 1))
                xsb = xab_sbs[xp]
                nc.tensor.matmul(pa[:], w1a, xsb[:, lsl], start=True,
                                 stop=True)
                nc.tensor.matmul(pb[:], w1b, xsb[:, lsl], start=True,
                                 stop=True)
                rb = rbp.tile([128, 512], BF, tag="rb")
                nc.scalar.activation(rb[:], pb[:], Relu)
                sl = slice(512 * ch, 512 * (ch + 1))
                nc.vector.scalar_tensor_tensor(fact[:, sl], pa[:], 0.0,
                                               rb[:], op0=mx, op1=mult)

            sels = {}

            def stage2a(ch):
                t0 = 4 * ch
                pb2 = ps2.tile([128, 4, 64], F32, tag="p2")
                for k in range(4):
                    t = t0 + k
                    nc.tensor.matmul(pb2[:, k, :],
                                     fact[:, 128 * t:128 * (t + 1)],
                                     w2s[tile_i[t]], start=True, stop=True)
                sel = selp.tile([128, 4, 64], BF, tag="sel")
                mp, mo = piece_of(t0, MS)
                msl = msel_sbs[mp]
                if ch >= NCH - 5:
                    # drain: stage-1 consumers are done, DVE is free — mult
                    # straight from PSUM, skipping the copy+Pool hops
                    nc.vector.tensor_tensor(sel[:, :, :], pb2[:, :, :],
                                            msl[:, mo:mo + 4, :], mult)
                else:
                    # split the select: DVE mults tiles 0-1 straight from
                    # PSUM; ACT copies tiles 2-3 to SBUF (Pool cannot touch
                    # PSUM) and Pool mults them. Balances all three engines
                    # under the PE cadence.
                    nc.vector.tensor_tensor(sel[:, 0:2, :], pb2[:, 0:2, :],
                                            msl[:, mo:mo + 2, :], mult)
                    pc = selp.tile([128, 2, 64], BF, tag="pc")
                    nc.scalar.activation(pc[:, :, :], pb2[:, 2:4, :], Copy)
                    nc.gpsimd.tensor_tensor(sel[:, 2:4, :], pc[:, :, :],
                                            msl[:, mo + 2:mo + 4, :], mult)
                sels[ch] = sel

            def stage2b(ch):
                t0 = 4 * ch
                sel = sels.pop(ch)
                for k in range(4):
                    t = t0 + k
                    q = tile_q[t]
                    h, hq = q // 2, q % 2
                    pn = pns[h]
                    op, oo = piece_of(t, OS)
                    osb = oneh_sbs[op]
                    # start=True only on the first write to each pn bank: it
                    # marks the whole 2KB zero-region pending-zero; each later
                    # region's first touch then auto-zeroes (flag persists).
                    nc.tensor.matmul(
                        pn[:, hq, :, :],
                        osb[:, 128 * oo:128 * (oo + 1)],
                        sel[:, k, :],
                        start=(t == 0 or t == bankB_first), stop=False,
                        skip_group_check=True)
                    if t == 0 or t == bankB_first:
                        # bias for this bank's two q windows: first touch of
                        # each window overwrites (pending-zero), later segsums
                        # accumulate on top
                        for q2 in (2 * h, 2 * h + 1):
                            nc.tensor.matmul(pn[:, q2 % 2, :, :], cmats[q2],
                                             bmat, start=False, stop=False,
                                             skip_group_check=True)
                    if t == reduceA_t or t == reduceB_t:
                        # bank complete: tau-reduce folds the per-pair output-
                        # type selection; stream this half's result out now
                        nc.vector.tensor_reduce(oc[:, 2 * h:2 * h + 2, :],
                                                pn[:, :, :, :],
                                                axis=mybir.AxisListType.X,
                                                op=add)
                        nc.sync.dma_start(out[:, 32 * h:32 * (h + 1)],
                                          oc[:, 2 * h:2 * h + 2, :])

            # stage2a (pb2 matmuls + copy + mult) lags stage1 by 2 chunks;
            # stage2b (segsum matmuls) lags 5, so the PE stream never sits
            # behind the just-emitted copy->mult chain of the same block.
            for ch in range(NCH + 5):
                if ch < NCH:
                    stage1(ch)
                if 2 <= ch < NCH + 2:
                    stage2a(ch - 2)
                if ch >= 5:
                    stage2b(ch - 5)

    nc.compile()
    return nc


def _remap(edges):
    """Assign nodes to (core, q, row) so (q,i) incidence groups pack tightly:
    top-degree nodes concentrate in q=3 (more tiles), q0..2 stay <=4 tiles.
    Returns node_ids [8,4,128] (node index per slot) and per-node lookups."""
    deg = np.zeros((N, 3), np.int64)
    for i in range(ORDER):
        deg[:, i] = np.bincount(edges[:, i], minlength=N)
    tot = deg.sum(1)
    order = np.argsort(-tot, kind="stable")
    node2bin = np.zeros(N, np.int64)
    heavy, light = order[:1024], order[1024:]
    for j, n in enumerate(heavy):      # snake over 8 cores, q=3
        c = j % 16
        c = c if c < 8 else 15 - c
        node2bin[n] = c * 4 + 3
    bins = [(c, q) for q in range(3) for c in range(8)]
    nb = len(bins)
    for j, n in enumerate(light):      # snake over 24 (c, q<3) bins
        k = j % (2 * nb)
        k = k if k < nb else 2 * nb - 1 - k
        c, q = bins[k]
        node2bin[n] = c * 4 + q
    node_ids = np.zeros((NCORES, 4, 128), np.int64)
    owner_of = np.zeros(N, np.int64)
    q_of = np.zeros(N, np.int64)
    r_of = np.zeros(N, np.int64)
    for b in range(32):
        ns = np.nonzero(node2bin == b)[0]
        assert len(ns) == 128
        c, q = b // 4, b % 4
        node_ids[c, q, :] = ns
        owner_of[ns] = c
        q_of[ns] = q
        r_of[ns] = np.arange(128)
    return node_ids, owner_of, q_of, r_of


def _prep_inputs(nodes, bp_params, bp_bias, ho_params, ho_bias, edges,
                 edge_types):
    nodes = np.asarray(nodes, np.float32)
    bp_params = np.asarray(bp_params, np.float32)
    bp_bias = np.asarray(bp_bias, np.float32)
    ho_params = np.asarray(ho_params, np.float32)
    ho_bias = np.asarray(ho_bias, np.float32)
    edges = np.asarray(edges, np.int64)
    edge_types = np.asarray(edge_types, np.int64)

    nodes_b = nodes.astype(bf16)
    node_ids, owner_of, q_of, r_of = _remap(edges)

    owner = owner_of[edges]   # [E, 3]
    q = q_of[edges]
    r = r_of[edges]

    group_lists = {}
    gmax = np.zeros((4, ORDER), np.int64)
    for c in range(NCORES):
        for i in range(ORDER):
            sel_c = owner[:, i] == c
            for qq in range(4):
                es = np.nonzero(sel_c & (q[:, i] == qq))[0]
                group_lists[(c, qq, i)] = es
                gmax[qq, i] = max(gmax[qq, i], len(es))
    tg = np.ceil(gmax / 128).astype(int)   # tiles per (q, i) group
    base = np.zeros((4, ORDER), np.int64)  # first tile of each group
    tile_q, tile_i = [], []
    for qq in range(4):
        for i in range(ORDER):
            base[qq, i] = len(tile_q)
            tile_q += [qq] * tg[qq, i]
            tile_i += [i] * tg[qq, i]
    while len(tile_q) % 4:                 # pad to whole 512-col chunks
        tile_q.append(3)
        tile_i.append(2)
    tile_q, tile_i = tuple(tile_q), tuple(tile_i)
    T = len(tile_q)
    L = 128 * T
    XP, MTS, OTS = _pieces(T)

    # packed weight tables (shared across cores)
    wpack = np.zeros((128, 448), np.float32)
    cpack0 = np.zeros((12, 576), np.float32)
    for p in range(NP_):
        wpack[13 * p:13 * p + 13, 0:128] = bp_params[p]
        wpack[52 + p, 0:128] = bp_bias[p, 0, :]
        wpack[64 + 13 * p:64 + 13 * p + 13, 128:256] = bp_params[p]
        wpack[116 + p, 128:256] = bp_bias[p, 0, :]
    # w2 columns in (d, tau) order
    for i in range(ORDER):
        for p in range(NP_):
            for dd in range(D):
                wpack[:, 256 + 64 * i + 4 * dd + p] = ho_params[i, p, :, dd]
    # bmat: bias rows per group g=4i+p, tau=0 slot (col 4d)
    for i in range(ORDER):
        for p in range(NP_):
            for dd in range(D):
                cpack0[4 * i + p, 512 + 4 * dd] = ho_bias[i, p, 0, dd]

    in_maps = []
    for c in range(NCORES):
        xab = np.zeros((128, L), np.float32)
        msel = np.zeros((128, T * 64), np.float32)
        oneh = np.zeros((128, T * 128), np.float32)
        cp = cpack0.copy()
        for qq in range(4):
            for i in range(ORDER):
                es = group_lists[(c, qq, i)]
                k = np.arange(len(es))
                x = 128 * base[qq, i] + k
                t_arr = x // 128
                lane = x % 128
                a, b = (i + 1) % 3, (i + 2) % 3
                ta_t = edge_types[es, a]
                tb_t = edge_types[es, b]
                fa = nodes_b[edges[es, a]].astype(np.float32)  # [m, 13]
                fb = nodes_b[edges[es, b]].astype(np.float32)
                for dd in range(D):
                    xab[13 * ta_t + dd, x] = fa[:, dd]
                    xab[64 + 13 * tb_t + dd, x] = fb[:, dd]
                xab[52 + ta_t, x] = 1.0
                xab[116 + tb_t, x] = 1.0
                p_e = edge_types[es, i]
                for dd in range(D):
                    msel[lane, 64 * t_arr + 4 * dd + p_e] = 1.0
                oneh[lane, 128 * t_arr + r[es, i]] = 1.0
                # bias count matrix: cmat[g=4i+p, q, node_row]
                cnt = np.bincount(r[es, i] + 128 * p_e, minlength=128 * NP_)
                for p in range(NP_):
                    cp[4 * i + p, 128 * qq:128 * (qq + 1)] += \
                        cnt[128 * p:128 * (p + 1)]
        xab_b = xab.astype(bf16)
        oneh_b = oneh.astype(f8)
        msel_b = msel.astype(f8)
        m = {"wpack": wpack.astype(bf16), "cpack": cp.astype(bf16)}
        xs = np.cumsum([0] + XP)
        for j in range(len(XP)):
            m[f"xab{j}"] = xab_b[:, 512 * xs[j]:512 * xs[j + 1]]
        ms = np.cumsum([0] + MTS)
        for j in range(len(MTS)):
            m[f"msel{j}"] = msel_b[:, 64 * ms[j]:64 * ms[j + 1]]
        os_ = np.cumsum([0] + OTS)
        for j in range(len(OTS)):
            m[f"oneh{j}"] = oneh_b[:, 128 * os_[j]:128 * os_[j + 1]]
        in_maps.append(m)
    return in_maps, (tile_q, tile_i), node_ids


def kernel(nodes, bp_params, bp_bias, ho_params, ho_bias, edges, edge_types,
           atoms=None, atom_edges=None, _run_kwargs=None):
    from concourse.bass_utils import run_bass_kernel_spmd

    in_maps, key, node_ids = _prep_inputs(nodes, bp_params, bp_bias,
                                          ho_params, ho_bias, edges,
                                          edge_types)
    if key not in _COMPILED:
        _COMPILED[key] = _build_program(*key)
    nc = _COMPILED[key]

    res = run_bass_kernel_spmd(nc, in_maps, core_ids=list(range(NCORES)),
                               **(_run_kwargs or {}))
    full = np.zeros((N, D), np.float32)
    for c in range(NCORES):
        oc = res.results[c]["out"]  # [128, 64] = [128r, 4q, 16d]
        vals = np.asarray(oc).reshape(128, 4, 16).transpose(1, 0, 2)[:, :, :D]
        full[node_ids[c].reshape(-1)] = vals.reshape(512, D)
    kernel._last_result = res
    return full


# revision 33
# speedup vs baseline: 1.0044x; 1.0044x over previous
"""BPNet GNN message-passing kernel for 8 Trainium2 NeuronCores.

Strategy (forced by this image: no extended-GPSIMD ucode, no indirect DMA —
both crash the device, verified experimentally; only static DMA + PE + DVE/ACT
work):
  - Node-sharded output with a data-driven node->(core, chunk) REMAP: nodes
    are redistributed so that per-(chunk q, slot i) incidence-group sizes pack
    tightly into 128-pair tiles (heavy-degree nodes concentrated in q=3, which
    gets more tiles; q=0..2 groups fit 4 tiles each). Every (edge,slot) pair
    is routed (host-side marshaling) to the owner core of its target node.
  - Host packs, per core, a position stream of pairs grouped by (q, i) with
    per-group tile counts (shared across cores). All per-pair data is baked
    into dense device tensors:
      xab  [128, L]  bf16 : one-hot-placed gathered node features + type
                            indicator for the pair's two COMPANION slots
                            (K-dim one-hot folds the per-edge weight selection
                            and bias into one fixed matmul).
      msel [128,T*64] fp8  : per-pair output-type selection mask, (d,tau)
                            column order (col 4d+tau); 0/1 exact in fp8
      oneh [128,T*128]fp8  : per-tile one-hot incidence (lane -> node row)
      cmat/bmat           : per-(slot,type)-group incidence counts + ho_bias
                            rows; bias enters via tiny matmuls.
  - Device pipeline, chunk ch = 512 pair-columns = 4 tiles:
      PE:   pa = w1a^T @ xab[ch], pb = w1b^T @ xab[ch]          (psum)
      ACT:  rb = relu(pb)                                        (sbuf bf16)
      DVE:  fact[ch] = max(pa,0) * rb    (fused scalar_tensor_tensor)
      PE:   pb2[k] = fact_t^T @ w2[:, i_t, :]  (4 tiles -> one psum block)
      ACT/DVE: pc = copy(pb2)  (Pool cannot touch PSUM; alternate engines)
      POOL: sel = pc * msel[ch]                                  (sbuf bf16)
      PE:   pn[:, q, :, :] += oneh_t^T @ sel_t   (psum wide accumulate over
            (d,tau); type-select deferred to a per-bank final tau-reduce)
      PE:   pn[:, q, :, :] += cmat_q^T @ bmat    (bias via group counts)
      DVE:  oc = reduce_tau(pn)  -> [128, 64] f32 -> DMA out (per pn bank)
    Stage-2a/2b emission lags stage-1 by 2/5 chunks so the PE stream never
    waits on the copy->mult chain (PE p-state: clock doubles only after ~3us
    of continuous busy).
  - Output per core: [128, 64] f32 = remapped nodes [row, 16q+d]; unshard =
    permutation scatter on host.
"""

import numpy as np
import ml_dtypes

N, E, ORDER, D, RANK = 4096, 16384, 3, 13, 128
NP_ = ORDER + 1  # 4 types
NCORES = 8

bf16 = ml_dtypes.bfloat16
f8 = ml_dtypes.float8_e4m3

_COMPILED = {}  # (tile_q, tile_i) -> nc


def _pieces(T):
    """Shared host/device derivation of DMA piece splits for a T-tile layout.
    Returns (xab piece sizes in chunks, msel piece sizes in tiles (4-aligned),
    oneh piece sizes in tiles)."""
    NCH = T // 4
    a = max(1, (NCH - 2) // 3)
    xp = [2, a, a, NCH - 2 - 2 * a]
    mt0 = ((T // 2) + 3) // 4 * 4
    mts = [mt0, T - mt0]
    ot = T // 4
    ots = [ot, ot, ot, T - 3 * ot]
    return xp, mts, ots


def _build_program(tile_q, tile_i):
    import concourse.bacc as bacc
    import concourse.tile as tile
    from concourse import mybir

    T = len(tile_q)
    L = 128 * T
    NCH = T // 4
    XP, MTS, OTS = _pieces(T)
    XS = np.cumsum([0] + XP).tolist()      # xab piece start chunk
    MS = np.cumsum([0] + MTS).tolist()     # msel piece start tile
    OS = np.cumsum([0] + OTS).tolist()     # oneh piece start tile
    bankB_first = min(t for t in range(T) if tile_q[t] >= 2)
    reduceA_t = max(t for t in range(T) if tile_q[t] <= 1)
    reduceB_t = T - 1

    def piece_of(t, starts):
        j = max(k for k in range(len(starts) - 1) if starts[k] <= t)
        return j, t - starts[j]

    nc = bacc.Bacc("TRN2", target_bir_lowering=False, debug=False,
                   num_devices=NCORES, enable_partition_id=False)
    BF, F32 = mybir.dt.bfloat16, mybir.dt.float32
    F8 = mybir.dt.float8e4
    Relu = mybir.ActivationFunctionType.Relu
    Copy = mybir.ActivationFunctionType.Copy
    mx, mult, add = (mybir.AluOpType.max, mybir.AluOpType.mult,
                     mybir.AluOpType.add)

    wpack = nc.dram_tensor("wpack", [128, 448], BF, kind="ExternalInput").ap()
    cpack = nc.dram_tensor("cpack", [12, 576], BF, kind="ExternalInput").ap()
    xabs = [nc.dram_tensor(f"xab{j}", [128, 512 * XP[j]], BF,
                           kind="ExternalInput").ap() for j in range(len(XP))]
    msels = [nc.dram_tensor(f"msel{j}", [128, MTS[j] * 64], F8,
                            kind="ExternalInput").ap()
             for j in range(len(MTS))]
    onehs = [nc.dram_tensor(f"oneh{j}", [128, OTS[j] * 128], F8,
                            kind="ExternalInput").ap()
             for j in range(len(OTS))]
    out = nc.dram_tensor("out", [128, 64], F32, kind="ExternalOutput").ap()

    with tile.TileContext(nc) as tc:
        with tc.tile_pool(name="inp", bufs=1) as inp, \
             tc.tile_pool(name="work", bufs=1) as work, \
             tc.tile_pool(name="rbp", bufs=3) as rbp, \
             tc.tile_pool(name="selp", bufs=10) as selp, \
             tc.tile_pool(name="ps1", bufs=4, space="PSUM") as ps1, \
             tc.tile_pool(name="ps2", bufs=3, space="PSUM") as ps2, \
             tc.tile_pool(name="psn", bufs=1, space="PSUM") as psn:
            wpack_sb = inp.tile([128, 448], BF, tag="wpack")
            cpack_sb = inp.tile([12, 576], BF, tag="cpack")
            xab_sbs = [inp.tile([128, 512 * XP[j]], BF, tag=f"xab{j}",
                                name=f"xab{j}_sb") for j in range(len(XP))]
            msel_sbs = [inp.tile([128, MTS[j], 64], F8, tag=f"msel{j}",
                                 name=f"msel{j}_sb") for j in range(len(MTS))]
            oneh_sbs = [inp.tile([128, OTS[j] * 128], F8, tag=f"oneh{j}",
                                 name=f"oneh{j}_sb") for j in range(len(OTS))]

            # DMA issue order == expected consumption order, all on sync so
            # per-queue FIFO keeps bandwidth priority aligned with need time.
            nc.sync.dma_start(wpack_sb[:], wpack[:])
            nc.sync.dma_start(xab_sbs[0][:], xabs[0][:])
            nc.sync.dma_start(xab_sbs[1][:], xabs[1][:])
            nc.sync.dma_start(oneh_sbs[0][:], onehs[0][:])
            nc.sync.dma_start(cpack_sb[:], cpack[:])
            nc.sync.dma_start(msel_sbs[0][:, :, :], msels[0][:])
            nc.sync.dma_start(oneh_sbs[1][:], onehs[1][:])
            nc.sync.dma_start(xab_sbs[2][:], xabs[2][:])
            nc.sync.dma_start(oneh_sbs[2][:], onehs[2][:])
            nc.sync.dma_start(msel_sbs[1][:, :, :], msels[1][:])
            nc.sync.dma_start(xab_sbs[3][:], xabs[3][:])
            nc.sync.dma_start(oneh_sbs[3][:], onehs[3][:])

            w1a = wpack_sb[:, 0:128]
            w1b = wpack_sb[:, 128:256]
            # w2 columns in (d, tau) order: col 4d+tau
            w2s = [wpack_sb[:, 256 + 64 * i:256 + 64 * (i + 1)]
                   for i in range(3)]
            cmats = [cpack_sb[:, 128 * q:128 * (q + 1)] for q in range(4)]
            bmat = cpack_sb[:, 512:576]

            fact = work.tile([128, L], BF, tag="fact")
            oc = work.tile([128, 4, 16], F32, tag="oc")
            # node psum split across 2 banks (q0,q1 | q2,q3) so the per-half
            # tau-reduce never blocks later segsum matmuls (no WAR on PE)
            pns = [psn.tile([128, 2, 16, 4], F32, tag="pn", name=f"pn{h}")
                   for h in range(2)]

            def stage1(ch):
                pa = ps1.tile([128, 512], F32, tag="p1")
                pb = ps1.tile([128, 512], F32, tag="p1")
                xp, xo = piece_of(ch, XS)
                lsl = slice(512 * xo, 512 * (xo + 1))
                xsb = xab_sbs[xp]
                nc.tensor.matmul(pa[:], w1a, xsb[:, lsl], start=True,
                                 stop=True)
                nc.tensor.matmul(pb[:], w1b, xsb[:, lsl], start=True,
                                 stop=True)
                rb = rbp.tile([128, 512], BF, tag="rb")
                nc.scalar.activation(rb[:], pb[:], Relu)
                sl = slice(512 * ch, 512 * (ch + 1))
                nc.vector.scalar_tensor_tensor(fact[:, sl], pa[:], 0.0,
                                               rb[:], op0=mx, op1=mult)

            sels = {}

            def stage2a(ch):
                t0 = 4 * ch
                pb2 = ps2.tile([128, 4, 64], F32, tag="p2")
                for k in range(4):
                    t = t0 + k
                    nc.tensor.matmul(pb2[:, k, :],
                                     fact[:, 128 * t:128 * (t + 1)],
                                     w2s[tile_i[t]], start=True, stop=True)
                sel = selp.tile([128, 4, 64], BF, tag="sel")
                mp, mo = piece_of(t0, MS)
                msl = msel_sbs[mp]
                if ch >= NCH - 5:
                    # drain: stage-1 consumers are done, DVE is free — mult
                    # straight from PSUM, skipping the copy+Pool hops
                    nc.vector.tensor_tensor(sel[:, :, :], pb2[:, :, :],
                                            msl[:, mo:mo + 4, :], mult)
                else:
                    # split the select: DVE mults tiles 0-2 straight from
                    # PSUM; ACT copies tile 3 to SBUF (Pool cannot touch
                    # PSUM) and Pool mults it. Balances all three engines
                    # under the PE cadence.
                    nc.vector.tensor_tensor(sel[:, 0:3, :], pb2[:, 0:3, :],
                                            msl[:, mo:mo + 3, :], mult)
                    pc = selp.tile([128, 1, 64], BF, tag="pc")
                    nc.scalar.activation(pc[:, :, :], pb2[:, 3:4, :], Copy)
                    nc.gpsimd.tensor_tensor(sel[:, 3:4, :], pc[:, :, :],
                                            msl[:, mo + 3:mo + 4, :], mult)
                sels[ch] = sel

            def stage2b(ch):
                t0 = 4 * ch
                sel = sels.pop(ch)
                for k in range(4):
                    t = t0 + k
                    q = tile_q[t]
                    h, hq = q // 2, q % 2
                    pn = pns[h]
                    op, oo = piece_of(t, OS)
                    osb = oneh_sbs[op]
                    # start=True only on the first write to each pn bank: it
                    # marks the whole 2KB zero-region pending-zero; each later
                    # region's first touch then auto-zeroes (flag persists).
                    nc.tensor.matmul(
                        pn[:, hq, :, :],
                        osb[:, 128 * oo:128 * (oo + 1)],
                        sel[:, k, :],
                        start=(t == 0 or t == bankB_first), stop=False,
                        skip_group_check=True)
                    if t == 0 or t == bankB_first:
                        # bias for this bank's two q windows: first touch of
                        # each window overwrites (pending-zero), later segsums
                        # accumulate on top
                        for q2 in (2 * h, 2 * h + 1):
                            nc.tensor.matmul(pn[:, q2 % 2, :, :], cmats[q2],
                                             bmat, start=False, stop=False,
                                             skip_group_check=True)
                    if t == reduceA_t or t == reduceB_t:
                        # bank complete: tau-reduce folds the per-pair output-
                        # type selection; stream this half's result out now
                        nc.vector.tensor_reduce(oc[:, 2 * h:2 * h + 2, :],
                                                pn[:, :, :, :],
                                                axis=mybir.AxisListType.X,
                                                op=add)
                        nc.sync.dma_start(out[:, 32 * h:32 * (h + 1)],
                                          oc[:, 2 * h:2 * h + 2, :])

            # stage2a (pb2 matmuls + copy + mult) lags stage1 by 2 chunks;
            # stage2b (segsum matmuls) lags 5, so the PE stream never sits
            # behind the just-emitted copy->mult chain of the same block.
            for ch in range(NCH + 5):
                if ch < NCH:
                    stage1(ch)
                if 2 <= ch < NCH + 2:
                    stage2a(ch - 2)
                if ch >= 5:
                    stage2b(ch - 5)

    nc.compile()
    return nc


def _remap(edges):
    """Assign nodes to (core, q, row) so (q,i) incidence groups pack tightly:
    top-degree nodes concentrate in q=3 (more tiles), q0..2 stay <=4 tiles.
    Returns node_ids [8,4,128] (node index per slot) and per-node lookups."""
    deg = np.zeros((N, 3), np.int64)
    for i in range(ORDER):
        deg[:, i] = np.bincount(edges[:, i], minlength=N)
    tot = deg.sum(1)
    order = np.argsort(-tot, kind="stable")
    node2bin = np.zeros(N, np.int64)
    heavy, light = order[:1024], order[1024:]
    for j, n in enumerate(heavy):      # snake over 8 cores, q=3
        c = j % 16
        c = c if c < 8 else 15 - c
        node2bin[n] = c * 4 + 3
    bins = [(c, q) for q in range(3) for c in range(8)]
    nb = len(bins)
    for j, n in enumerate(light):      # snake over 24 (c, q<3) bins
        k = j % (2 * nb)
        k = k if k < nb else 2 * nb - 1 - k
        c, q = bins[k]
        node2bin[n] = c * 4 + q
    node_ids = np.zeros((NCORES, 4, 128), np.int64)
    owner_of = np.zeros(N, np.int64)
    q_of = np.zeros(N, np.int64)
    r_of = np.zeros(N, np.int64)
    for b in range(32):
        ns = np.nonzero(node2bin == b)[0]
        assert len(ns) == 128
        c, q = b // 4, b % 4
        node_ids[c, q, :] = ns
        owner_of[ns] = c
        q_of[ns] = q
        r_of[ns] = np.arange(128)
    return node_ids, owner_of, q_of, r_of


def _prep_inputs(nodes, bp_params, bp_bias, ho_params, ho_bias, edges,
                 edge_types):
    nodes = np.asarray(nodes, np.float32)
    bp_params = np.asarray(bp_params, np.float32)
    bp_bias = np.asarray(bp_bias, np.float32)
    ho_params = np.asarray(ho_params, np.float32)
    ho_bias = np.asarray(ho_bias, np.float32)
    edges = np.asarray(edges, np.int64)
    edge_types = np.asarray(edge_types, np.int64)

    nodes_b = nodes.astype(bf16)
    node_ids, owner_of, q_of, r_of = _remap(edges)

    owner = owner_of[edges]   # [E, 3]
    q = q_of[edges]
    r = r_of[edges]

    group_lists = {}
    gmax = np.zeros((4, ORDER), np.int64)
    for c in range(NCORES):
        for i in range(ORDER):
            sel_c = owner[:, i] == c
            for qq in range(4):
                es = np.nonzero(sel_c & (q[:, i] == qq))[0]
                group_lists[(c, qq, i)] = es
                gmax[qq, i] = max(gmax[qq, i], len(es))
    tg = np.ceil(gmax / 128).astype(int)   # tiles per (q, i) group
    base = np.zeros((4, ORDER), np.int64)  # first tile of each group
    tile_q, tile_i = [], []
    for qq in range(4):
        for i in range(ORDER):
            base[qq, i] = len(tile_q)
            tile_q += [qq] * tg[qq, i]
            tile_i += [i] * tg[qq, i]
    while len(tile_q) % 4:                 # pad to whole 512-col chunks
        tile_q.append(3)
        tile_i.append(2)
    tile_q, tile_i = tuple(tile_q), tuple(tile_i)
    T = len(tile_q)
    L = 128 * T
    XP, MTS, OTS = _pieces(T)

    # packed weight tables (shared across cores)
    wpack = np.zeros((128, 448), np.float32)
    cpack0 = np.zeros((12, 576), np.float32)
    for p in range(NP_):
        wpack[13 * p:13 * p + 13, 0:128] = bp_params[p]
        wpack[52 + p, 0:128] = bp_bias[p, 0, :]
        wpack[64 + 13 * p:64 + 13 * p + 13, 128:256] = bp_params[p]
        wpack[116 + p, 128:256] = bp_bias[p, 0, :]
    # w2 columns in (d, tau) order
    for i in range(ORDER):
        for p in range(NP_):
            for dd in range(D):
                wpack[:, 256 + 64 * i + 4 * dd + p] = ho_params[i, p, :, dd]
    # bmat: bias rows per group g=4i+p, tau=0 slot (col 4d)
    for i in range(ORDER):
        for p in range(NP_):
            for dd in range(D):
                cpack0[4 * i + p, 512 + 4 * dd] = ho_bias[i, p, 0, dd]

    in_maps = []
    for c in range(NCORES):
        xab = np.zeros((128, L), np.float32)
        msel = np.zeros((128, T * 64), np.float32)
        oneh = np.zeros((128, T * 128), np.float32)
        cp = cpack0.copy()
        for qq in range(4):
            for i in range(ORDER):
                es = group_lists[(c, qq, i)]
                k = np.arange(len(es))
                x = 128 * base[qq, i] + k
                t_arr = x // 128
                lane = x % 128
                a, b = (i + 1) % 3, (i + 2) % 3
                ta_t = edge_types[es, a]
                tb_t = edge_types[es, b]
                fa = nodes_b[edges[es, a]].astype(np.float32)  # [m, 13]
                fb = nodes_b[edges[es, b]].astype(np.float32)
                for dd in range(D):
                    xab[13 * ta_t + dd, x] = fa[:, dd]
                    xab[64 + 13 * tb_t + dd, x] = fb[:, dd]
                xab[52 + ta_t, x] = 1.0
                xab[116 + tb_t, x] = 1.0
                p_e = edge_types[es, i]
                for dd in range(D):
                    msel[lane, 64 * t_arr + 4 * dd + p_e] = 1.0
                oneh[lane, 128 * t_arr + r[es, i]] = 1.0
                # bias count matrix: cmat[g=4i+p, q, node_row]
                cnt = np.bincount(r[es, i] + 128 * p_e, minlength=128 * NP_)
                for p in range(NP_):
                    cp[4 * i + p, 128 * qq:128 * (qq + 1)] += \
                        cnt[128 * p:128 * (p + 1)]
        xab_b = xab.astype(bf16)
        oneh_b = oneh.astype(f8)
        msel_b = msel.astype(f8)
        m = {"wpack": wpack.astype(bf16), "cpack": cp.astype(bf16)}
        xs = np.cumsum([0] + XP)
        for j in range(len(XP)):
            m[f"xab{j}"] = xab_b[:, 512 * xs[j]:512 * xs[j + 1]]
        ms = np.cumsum([0] + MTS)
        for j in range(len(MTS)):
            m[f"msel{j}"] = msel_b[:, 64 * ms[j]:64 * ms[j + 1]]
        os_ = np.cumsum([0] + OTS)
        for j in range(len(OTS)):
            m[f"oneh{j}"] = oneh_b[:, 128 * os_[j]:128 * os_[j + 1]]
        in_maps.append(m)
    return in_maps, (tile_q, tile_i), node_ids


def kernel(nodes, bp_params, bp_bias, ho_params, ho_bias, edges, edge_types,
           atoms=None, atom_edges=None, _run_kwargs=None):
    from concourse.bass_utils import run_bass_kernel_spmd

    in_maps, key, node_ids = _prep_inputs(nodes, bp_params, bp_bias,
                                          ho_params, ho_bias, edges,
                                          edge_types)
    if key not in _COMPILED:
        _COMPILED[key] = _build_program(*key)
    nc = _COMPILED[key]

    res = run_bass_kernel_spmd(nc, in_maps, core_ids=list(range(NCORES)),
                               **(_run_kwargs or {}))
    full = np.zeros((N, D), np.float32)
    for c in range(NCORES):
        oc = res.results[c]["out"]  # [128, 64] = [128r, 4q, 16d]
        vals = np.asarray(oc).reshape(128, 4, 16).transpose(1, 0, 2)[:, :, :D]
        full[node_ids[c].reshape(-1)] = vals.reshape(512, D)
    kernel._last_result = res
    return full


# revision 35
# speedup vs baseline: 1.0718x; 1.0671x over previous
"""BPNet GNN message-passing kernel for 8 Trainium2 NeuronCores.

Strategy (forced by this image: no extended-GPSIMD ucode, no indirect DMA —
both crash the device, verified experimentally; only static DMA + PE + DVE/ACT
work):
  - Node-sharded output with a data-driven node->(core, chunk) REMAP: nodes
    are redistributed so that per-(chunk q, slot i) incidence-group sizes pack
    tightly into 128-pair tiles (heavy-degree nodes concentrated in q=3, which
    gets more tiles; q=0..2 groups fit 4 tiles each). Every (edge,slot) pair
    is routed (host-side marshaling) to the owner core of its target node.
  - Host packs, per core, a position stream of pairs grouped by (q, i) with
    per-group tile counts (shared across cores). All per-pair data is baked
    into dense device tensors:
      xab  [128, L]  bf16 : one-hot-placed gathered node features + type
                            indicator for the pair's two COMPANION slots
                            (K-dim one-hot folds the per-edge weight selection
                            and bias into one fixed matmul).
      msel [128,T*64] fp8  : per-pair output-type selection mask, (d,tau)
                            column order (col 4d+tau); 0/1 exact in fp8
      oneh [128,T*128]fp8  : per-tile one-hot incidence (lane -> node row)
      cmat/bmat           : per-(slot,type)-group incidence counts + ho_bias
                            rows; bias enters via tiny matmuls.
  - Device pipeline, chunk ch = 512 pair-columns = 4 tiles:
      PE:   pa = w1a^T @ xab[ch], pb = w1b^T @ xab[ch]          (psum)
      ACT:  rb = relu(pb)                                        (sbuf bf16)
      DVE:  fact[ch] = max(pa,0) * rb    (fused scalar_tensor_tensor)
      PE:   pb2[k] = fact_t^T @ w2[:, i_t, :]  (4 tiles -> one psum block)
      DVE:  sel[0:3] = pb2 * msel  (straight from PSUM)
      ACT+POOL: sel[3] via copy + mult  (Pool cannot touch PSUM; the split
            keeps all three engines under the PE cadence)
      PE:   pn[:, q, :, :] += oneh_t^T @ sel_t   (psum wide accumulate over
            (d,tau); type-select deferred to a per-bank final tau-reduce)
      PE:   pn[:, q, :, :] += cmat_q^T @ bmat    (bias via group counts)
      DVE:  oc = reduce_tau(pn)  -> [128, 64] f32 -> DMA out (per pn bank)
    Stage-2a/2b emission lags stage-1 by 2/5 chunks so the PE stream never
    waits on the copy->mult chain (PE p-state: clock doubles only after ~3us
    of continuous busy).
  - Output per core: [128, 64] f32 = remapped nodes [row, 16q+d]; unshard =
    permutation scatter on host.
"""

import numpy as np
import ml_dtypes

N, E, ORDER, D, RANK = 4096, 16384, 3, 13, 128
NP_ = ORDER + 1  # 4 types
NCORES = 8

bf16 = ml_dtypes.bfloat16
f8 = ml_dtypes.float8_e4m3

_COMPILED = {}  # (tile_q, tile_i) -> nc


def _pieces(T):
    """Shared host/device derivation of DMA piece splits for a T-tile layout.
    Returns (xab piece sizes in chunks, msel piece sizes in tiles (4-aligned),
    oneh piece sizes in tiles)."""
    NCH = T // 4
    a = max(1, (NCH - 2) // 3)
    xp = [2, a, a, NCH - 2 - 2 * a]
    mt0 = ((T // 2) + 3) // 4 * 4
    mts = [mt0, T - mt0]
    ot = T // 4
    ots = [ot, ot, ot, T - 3 * ot]
    return xp, mts, ots


def _build_program(tile_q, tile_i):
    import concourse.bacc as bacc
    import concourse.tile as tile
    from concourse import mybir

    T = len(tile_q)
    L = 128 * T
    NCH = T // 4
    XP, MTS, OTS = _pieces(T)
    XS = np.cumsum([0] + XP).tolist()      # xab piece start chunk
    MS = np.cumsum([0] + MTS).tolist()     # msel piece start tile
    OS = np.cumsum([0] + OTS).tolist()     # oneh piece start tile
    bankB_first = min(t for t in range(T) if tile_q[t] >= 2)
    reduceA_t = max(t for t in range(T) if tile_q[t] <= 1)
    reduceB_t = T - 1

    def piece_of(t, starts):
        j = max(k for k in range(len(starts) - 1) if starts[k] <= t)
        return j, t - starts[j]

    nc = bacc.Bacc("TRN2", target_bir_lowering=False, debug=False,
                   num_devices=NCORES, enable_partition_id=False)
    BF, F32 = mybir.dt.bfloat16, mybir.dt.float32
    F8 = mybir.dt.float8e4
    Relu = mybir.ActivationFunctionType.Relu
    Copy = mybir.ActivationFunctionType.Copy
    mx, mult, add = (mybir.AluOpType.max, mybir.AluOpType.mult,
                     mybir.AluOpType.add)

    wpack = nc.dram_tensor("wpack", [128, 448], BF, kind="ExternalInput").ap()
    cpack = nc.dram_tensor("cpack", [12, 576], BF, kind="ExternalInput").ap()
    xabs = [nc.dram_tensor(f"xab{j}", [128, 512 * XP[j]], BF,
                           kind="ExternalInput").ap() for j in range(len(XP))]
    msels = [nc.dram_tensor(f"msel{j}", [128, MTS[j] * 52], F8,
                            kind="ExternalInput").ap()
             for j in range(len(MTS))]
    onehs = [nc.dram_tensor(f"oneh{j}", [128, OTS[j] * 128], F8,
                            kind="ExternalInput").ap()
             for j in range(len(OTS))]
    out = nc.dram_tensor("out", [128, 52], F32, kind="ExternalOutput").ap()

    with tile.TileContext(nc) as tc:
        with tc.tile_pool(name="inp", bufs=1) as inp, \
             tc.tile_pool(name="work", bufs=1) as work, \
             tc.tile_pool(name="rbp", bufs=3) as rbp, \
             tc.tile_pool(name="selp", bufs=10) as selp, \
             tc.tile_pool(name="ps1", bufs=4, space="PSUM") as ps1, \
             tc.tile_pool(name="ps2", bufs=3, space="PSUM") as ps2, \
             tc.tile_pool(name="psn", bufs=1, space="PSUM") as psn:
            wpack_sb = inp.tile([128, 448], BF, tag="wpack")
            cpack_sb = inp.tile([12, 576], BF, tag="cpack")
            xab_sbs = [inp.tile([128, 512 * XP[j]], BF, tag=f"xab{j}",
                                name=f"xab{j}_sb") for j in range(len(XP))]
            msel_sbs = [inp.tile([128, MTS[j], 52], F8, tag=f"msel{j}",
                                 name=f"msel{j}_sb") for j in range(len(MTS))]
            oneh_sbs = [inp.tile([128, OTS[j] * 128], F8, tag=f"oneh{j}",
                                 name=f"oneh{j}_sb") for j in range(len(OTS))]

            # DMA issue order == expected consumption order, all on sync so
            # per-queue FIFO keeps bandwidth priority aligned with need time.
            nc.sync.dma_start(wpack_sb[:], wpack[:])
            nc.sync.dma_start(xab_sbs[0][:], xabs[0][:])
            nc.sync.dma_start(xab_sbs[1][:], xabs[1][:])
            nc.sync.dma_start(oneh_sbs[0][:], onehs[0][:])
            nc.sync.dma_start(cpack_sb[:], cpack[:])
            nc.sync.dma_start(msel_sbs[0][:, :, :], msels[0][:])
            nc.sync.dma_start(oneh_sbs[1][:], onehs[1][:])
            nc.sync.dma_start(xab_sbs[2][:], xabs[2][:])
            nc.sync.dma_start(oneh_sbs[2][:], onehs[2][:])
            nc.sync.dma_start(msel_sbs[1][:, :, :], msels[1][:])
            nc.sync.dma_start(xab_sbs[3][:], xabs[3][:])
            nc.sync.dma_start(oneh_sbs[3][:], onehs[3][:])

            w1a = wpack_sb[:, 0:128]
            w1b = wpack_sb[:, 128:256]
            # w2 columns in (d, tau) order: col 4d+tau, d < 13 -> 52 wide
            w2s = [wpack_sb[:, 256 + 52 * i:256 + 52 * (i + 1)]
                   for i in range(3)]
            cmats = [cpack_sb[:, 128 * q:128 * (q + 1)] for q in range(4)]
            bmat = cpack_sb[:, 512:564]

            fact = work.tile([128, L], BF, tag="fact")
            oc = work.tile([128, 4, 13], F32, tag="oc")
            # node psum split across 2 banks (q0,q1 | q2,q3) so the per-half
            # tau-reduce never blocks later segsum matmuls (no WAR on PE)
            pns = [psn.tile([128, 2, 13, 4], F32, tag="pn", name=f"pn{h}")
                   for h in range(2)]

            def stage1(ch):
                pa = ps1.tile([128, 512], F32, tag="p1")
                pb = ps1.tile([128, 512], F32, tag="p1")
                xp, xo = piece_of(ch, XS)
                lsl = slice(512 * xo, 512 * (xo + 1))
                xsb = xab_sbs[xp]
                nc.tensor.matmul(pa[:], w1a, xsb[:, lsl], start=True,
                                 stop=True)
                nc.tensor.matmul(pb[:], w1b, xsb[:, lsl], start=True,
                                 stop=True)
                rb = rbp.tile([128, 512], BF, tag="rb")
                nc.scalar.activation(rb[:], pb[:], Relu)
                sl = slice(512 * ch, 512 * (ch + 1))
                nc.vector.scalar_tensor_tensor(fact[:, sl], pa[:], 0.0,
                                               rb[:], op0=mx, op1=mult)

            sels = {}

            def stage2a(ch):
                t0 = 4 * ch
                pb2 = ps2.tile([128, 4, 52], F32, tag="p2")
                for k in range(4):
                    t = t0 + k
                    nc.tensor.matmul(pb2[:, k, :],
                                     fact[:, 128 * t:128 * (t + 1)],
                                     w2s[tile_i[t]], start=True, stop=True)
                sel = selp.tile([128, 4, 52], BF, tag="sel")
                mp, mo = piece_of(t0, MS)
                msl = msel_sbs[mp]
                if ch >= NCH - 5:
                    # drain: stage-1 consumers are done, DVE is free — mult
                    # straight from PSUM, skipping the copy+Pool hops
                    nc.vector.tensor_tensor(sel[:, :, :], pb2[:, :, :],
                                            msl[:, mo:mo + 4, :], mult)
                else:
                    # split the select: DVE mults tiles 0-2 straight from
                    # PSUM; ACT copies tile 3 to SBUF (Pool cannot touch
                    # PSUM) and Pool mults it. Balances all three engines
                    # under the PE cadence.
                    nc.vector.tensor_tensor(sel[:, 0:3, :], pb2[:, 0:3, :],
                                            msl[:, mo:mo + 3, :], mult)
                    pc = selp.tile([128, 1, 52], BF, tag="pc")
                    nc.scalar.activation(pc[:, :, :], pb2[:, 3:4, :], Copy)
                    nc.gpsimd.tensor_tensor(sel[:, 3:4, :], pc[:, :, :],
                                            msl[:, mo + 3:mo + 4, :], mult)
                sels[ch] = sel

            def stage2b(ch):
                t0 = 4 * ch
                sel = sels.pop(ch)
                for k in range(4):
                    t = t0 + k
                    q = tile_q[t]
                    h, hq = q // 2, q % 2
                    pn = pns[h]
                    op, oo = piece_of(t, OS)
                    osb = oneh_sbs[op]
                    # start=True only on the first write to each pn bank: it
                    # marks the whole 2KB zero-region pending-zero; each later
                    # region's first touch then auto-zeroes (flag persists).
                    nc.tensor.matmul(
                        pn[:, hq, :, :],
                        osb[:, 128 * oo:128 * (oo + 1)],
                        sel[:, k, :],
                        start=(t == 0 or t == bankB_first), stop=False,
                        skip_group_check=True)
                    if t == 0 or t == bankB_first:
                        # bias for this bank's two q windows: first touch of
                        # each window overwrites (pending-zero), later segsums
                        # accumulate on top
                        for q2 in (2 * h, 2 * h + 1):
                            nc.tensor.matmul(pn[:, q2 % 2, :, :], cmats[q2],
                                             bmat, start=False, stop=False,
                                             skip_group_check=True)
                    if t == reduceA_t or t == reduceB_t:
                        # bank complete: tau-reduce folds the per-pair output-
                        # type selection; stream this half's result out now
                        nc.vector.tensor_reduce(oc[:, 2 * h:2 * h + 2, :],
                                                pn[:, :, :, :],
                                                axis=mybir.AxisListType.X,
                                                op=add)
                        nc.sync.dma_start(out[:, 26 * h:26 * (h + 1)],
                                          oc[:, 2 * h:2 * h + 2, :])

            # stage2a (pb2 matmuls + copy + mult) lags stage1 by 2 chunks;
            # stage2b (segsum matmuls) lags 5, so the PE stream never sits
            # behind the just-emitted copy->mult chain of the same block.
            for ch in range(NCH + 5):
                if ch < NCH:
                    stage1(ch)
                if 2 <= ch < NCH + 2:
                    stage2a(ch - 2)
                if ch >= 5:
                    stage2b(ch - 5)

    nc.compile()
    return nc


def _remap(edges):
    """Assign nodes to (core, q, row) so (q,i) incidence groups pack tightly:
    top-degree nodes concentrate in q=3 (more tiles), q0..2 stay <=4 tiles.
    Returns node_ids [8,4,128] (node index per slot) and per-node lookups."""
    deg = np.zeros((N, 3), np.int64)
    for i in range(ORDER):
        deg[:, i] = np.bincount(edges[:, i], minlength=N)
    tot = deg.sum(1)
    order = np.argsort(-tot, kind="stable")
    node2bin = np.zeros(N, np.int64)
    heavy, light = order[:1024], order[1024:]
    for j, n in enumerate(heavy):      # snake over 8 cores, q=3
        c = j % 16
        c = c if c < 8 else 15 - c
        node2bin[n] = c * 4 + 3
    bins = [(c, q) for q in range(3) for c in range(8)]
    nb = len(bins)
    for j, n in enumerate(light):      # snake over 24 (c, q<3) bins
        k = j % (2 * nb)
        k = k if k < nb else 2 * nb - 1 - k
        c, q = bins[k]
        node2bin[n] = c * 4 + q
    node_ids = np.zeros((NCORES, 4, 128), np.int64)
    owner_of = np.zeros(N, np.int64)
    q_of = np.zeros(N, np.int64)
    r_of = np.zeros(N, np.int64)
    for b in range(32):
        ns = np.nonzero(node2bin == b)[0]
        assert len(ns) == 128
        c, q = b // 4, b % 4
        node_ids[c, q, :] = ns
        owner_of[ns] = c
        q_of[ns] = q
        r_of[ns] = np.arange(128)
    return node_ids, owner_of, q_of, r_of


def _prep_inputs(nodes, bp_params, bp_bias, ho_params, ho_bias, edges,
                 edge_types):
    nodes = np.asarray(nodes, np.float32)
    bp_params = np.asarray(bp_params, np.float32)
    bp_bias = np.asarray(bp_bias, np.float32)
    ho_params = np.asarray(ho_params, np.float32)
    ho_bias = np.asarray(ho_bias, np.float32)
    edges = np.asarray(edges, np.int64)
    edge_types = np.asarray(edge_types, np.int64)

    nodes_b = nodes.astype(bf16)
    node_ids, owner_of, q_of, r_of = _remap(edges)

    owner = owner_of[edges]   # [E, 3]
    q = q_of[edges]
    r = r_of[edges]

    group_lists = {}
    gmax = np.zeros((4, ORDER), np.int64)
    for c in range(NCORES):
        for i in range(ORDER):
            sel_c = owner[:, i] == c
            for qq in range(4):
                es = np.nonzero(sel_c & (q[:, i] == qq))[0]
                group_lists[(c, qq, i)] = es
                gmax[qq, i] = max(gmax[qq, i], len(es))
    tg = np.ceil(gmax / 128).astype(int)   # tiles per (q, i) group
    base = np.zeros((4, ORDER), np.int64)  # first tile of each group
    tile_q, tile_i = [], []
    for qq in range(4):
        for i in range(ORDER):
            base[qq, i] = len(tile_q)
            tile_q += [qq] * tg[qq, i]
            tile_i += [i] * tg[qq, i]
    while len(tile_q) % 4:                 # pad to whole 512-col chunks
        tile_q.append(3)
        tile_i.append(2)
    tile_q, tile_i = tuple(tile_q), tuple(tile_i)
    T = len(tile_q)
    L = 128 * T
    XP, MTS, OTS = _pieces(T)

    # packed weight tables (shared across cores)
    wpack = np.zeros((128, 448), np.float32)
    cpack0 = np.zeros((12, 576), np.float32)
    for p in range(NP_):
        wpack[13 * p:13 * p + 13, 0:128] = bp_params[p]
        wpack[52 + p, 0:128] = bp_bias[p, 0, :]
        wpack[64 + 13 * p:64 + 13 * p + 13, 128:256] = bp_params[p]
        wpack[116 + p, 128:256] = bp_bias[p, 0, :]
    # w2 columns in (d, tau) order, 52 wide per slot
    for i in range(ORDER):
        for p in range(NP_):
            for dd in range(D):
                wpack[:, 256 + 52 * i + 4 * dd + p] = ho_params[i, p, :, dd]
    # bmat: bias rows per group g=4i+p, tau=0 slot (col 4d)
    for i in range(ORDER):
        for p in range(NP_):
            for dd in range(D):
                cpack0[4 * i + p, 512 + 4 * dd] = ho_bias[i, p, 0, dd]

    in_maps = []
    for c in range(NCORES):
        xab = np.zeros((128, L), np.float32)
        msel = np.zeros((128, T * 52), np.float32)
        oneh = np.zeros((128, T * 128), np.float32)
        cp = cpack0.copy()
        for qq in range(4):
            for i in range(ORDER):
                es = group_lists[(c, qq, i)]
                k = np.arange(len(es))
                x = 128 * base[qq, i] + k
                t_arr = x // 128
                lane = x % 128
                a, b = (i + 1) % 3, (i + 2) % 3
                ta_t = edge_types[es, a]
                tb_t = edge_types[es, b]
                fa = nodes_b[edges[es, a]].astype(np.float32)  # [m, 13]
                fb = nodes_b[edges[es, b]].astype(np.float32)
                for dd in range(D):
                    xab[13 * ta_t + dd, x] = fa[:, dd]
                    xab[64 + 13 * tb_t + dd, x] = fb[:, dd]
                xab[52 + ta_t, x] = 1.0
                xab[116 + tb_t, x] = 1.0
                p_e = edge_types[es, i]
                for dd in range(D):
                    msel[lane, 52 * t_arr + 4 * dd + p_e] = 1.0
                oneh[lane, 128 * t_arr + r[es, i]] = 1.0
                # bias count matrix: cmat[g=4i+p, q, node_row]
                cnt = np.bincount(r[es, i] + 128 * p_e, minlength=128 * NP_)
                for p in range(NP_):
                    cp[4 * i + p, 128 * qq:128 * (qq + 1)] += \
                        cnt[128 * p:128 * (p + 1)]
        xab_b = xab.astype(bf16)
        oneh_b = oneh.astype(f8)
        msel_b = msel.astype(f8)
        m = {"wpack": wpack.astype(bf16), "cpack": cp.astype(bf16)}
        xs = np.cumsum([0] + XP)
        for j in range(len(XP)):
            m[f"xab{j}"] = xab_b[:, 512 * xs[j]:512 * xs[j + 1]]
        ms = np.cumsum([0] + MTS)
        for j in range(len(MTS)):
            m[f"msel{j}"] = msel_b[:, 52 * ms[j]:52 * ms[j + 1]]
        os_ = np.cumsum([0] + OTS)
        for j in range(len(OTS)):
            m[f"oneh{j}"] = oneh_b[:, 128 * os_[j]:128 * os_[j + 1]]
        in_maps.append(m)
    return in_maps, (tile_q, tile_i), node_ids


def kernel(nodes, bp_params, bp_bias, ho_params, ho_bias, edges, edge_types,
           atoms=None, atom_edges=None, _run_kwargs=None):
    from concourse.bass_utils import run_bass_kernel_spmd

    in_maps, key, node_ids = _prep_inputs(nodes, bp_params, bp_bias,
                                          ho_params, ho_bias, edges,
                                          edge_types)
    if key not in _COMPILED:
        _COMPILED[key] = _build_program(*key)
    nc = _COMPILED[key]

    res = run_bass_kernel_spmd(nc, in_maps, core_ids=list(range(NCORES)),
                               **(_run_kwargs or {}))
    full = np.zeros((N, D), np.float32)
    for c in range(NCORES):
        oc = res.results[c]["out"]  # [128, 52] = [128r, 4q, 13d]
        vals = np.asarray(oc).reshape(128, 4, 13).transpose(1, 0, 2)
        full[node_ids[c].reshape(-1)] = vals.reshape(512, D)
    kernel._last_result = res
    return full
